# revision 2
# baseline (speedup 1.0000x reference)
"""Trainium2 Bass kernel for nn_Distance_Sentences (retrieval_knn).

Computes out[b, i*O + o] = sum_k exp(-sum_n |proj[b,i,n,o] - proj[b,k,n,o]|)
with proj = x @ W^T, sharded over the batch (nsets) dim across 8 NeuronCores.

Self-contained: hardcodes shapes B=32, S=256, M=1024, N=O=32, 8 cores.
Inputs are pre-transposed/cast to bf16 on the host (x^T, W^T).
"""

import sys

for _p in ("/opt/trn_rl_repo", "/root/.axon_site/_ro/trn_rl_repo"):
    if _p not in sys.path:
        sys.path.insert(0, _p)

import re
import numpy as np

import concourse.bass as bass
import concourse.tile as tile
from concourse import mybir
from concourse import dve_ops
from concourse.dve_ops import DveOp
from concourse.dve_spec import (
    Spec,
    Src0,
    Src1,
    Zero,
    Leaf,
    scan,
    AluOp,
    Scan,
    _collect,
    _hoist_stream_invariant_ops,
    _build_placement,
    _assemble,
    _State,
    _Stage,
)
from concourse.dve_uop import InpSel, Trigger, DveOpSpec, N_LANES, N_STAGES, ENABLE

# ---------------------------------------------------------------- constants
B, S, M_DIM, N, O = 32, 256, 1024, 32, 32
NO = N * O  # 1024
NCORES = 8
SETS_PER_CORE = B // NCORES  # 4
IB = 4  # i's per partition-block -> 64 blocks per set
NBLK = S // IB  # 64
KCHUNK = 128  # pages per sub/abs/reduce chunk

# ------------------------------------------------- patched Tile final drain
# This walrus build rejects more than ONE sem-wait per instruction. Two
# patches: (1) the final drain emits individual wait_ge instructions;
# (2) a post-pass splits any multi-wait instruction by inserting
# EventSemaphore carrier instructions (one wait each) just before it.
_DRAIN_PATCHED = False

import bass_rust as _bass_rust


def _split_excess_waits(tc, ordered):
    nc = tc.nc
    for bbname, insts in ordered.items():
        out = []
        for inst in insts:
            si = inst.sync_info
            waits = list(si.on_wait) if si is not None else []
            if len(waits) > 1:
                # merge same-sem ge-waits (max value wins)
                merged = {}
                rest = []
                for w in waits:
                    if w.wait_mode == "sem-ge-imm" and w.wait_reg is None:
                        key = w.id
                        if key not in merged or merged[key].wait_value < w.wait_value:
                            merged[key] = w
                    else:
                        rest.append(w)
                waits = list(merged.values()) + rest
            if len(waits) > 1:
                keep = waits[-1]
                for w in waits[:-1]:
                    carrier = mybir.InstEventSemaphore(
                        name=nc.get_next_instruction_name(), ins=[], outs=[]
                    )
                    carrier.engine = inst.engine
                    carrier.sync_info = _bass_rust.SyncInfo(
                        on_wait=[w], on_update=[]
                    )
                    nc.register_instruction(carrier, overwrite=True)
                    out.append(carrier)
                inst.sync_info = _bass_rust.SyncInfo(
                    on_wait=[keep], on_update=list(si.on_update)
                )
            out.append(inst)
        ordered[bbname] = out
    return ordered


def _patch_tile_drain():
    global _DRAIN_PATCHED
    if _DRAIN_PATCHED:
        return
    _DRAIN_PATCHED = True

    orig_lower = tile.TileContext._lower_ordered_insts

    def lower_with_split(self, ordered):
        return orig_lower(self, _split_excess_waits(self, ordered))

    tile.TileContext._lower_ordered_insts = lower_with_split

    def patched(self, tick_clock, wait_clock):
        nc = self.nc
        gc = tick_clock.global_clock
        ticks = [int(x) for x in re.findall(r"\d+", repr(gc))]
        for proc, sem in self.sems.allocated().items():
            v = ticks[proc] if proc < len(ticks) else 0
            if v > 0:
                mult = 16 if "DMA" in sem.name else 1
                nc.sync.wait_ge(sem, v * mult)
        nc.sync.drain()
        nc.all_engine_barrier()
        popped = nc._tile_sem_poison_stack.pop()
        assert popped is self._sem_poison
        nc.clear_and_free_semaphores(list(self.sems.allocated().values()))
        nc.all_engine_barrier()

    tile.TileContext._drain_and_barrier = patched


# ------------------------------------- hijacked segmented sum-reduce (0x42)
# Row 0x42 (TENSOR_REDUCE) is overridden with a segmented sum program:
#   out[p, page] = sum_j in0[p, page, j]   (reset per page, write at page end)
# Mode variants are provided for every perf mode the engine may select.
# NOTE: stock nc.vector.tensor_reduce must NOT be used in this kernel.
REDUCE_OP_NAME = "SEG_SUM_REDUCE_ANT"
REDUCE_ROW = 0x42

_S0H = Leaf(InpSel.SRC_0_HI)
_S1H = Leaf(InpSel.SRC_1_HI)


def _build_variant(expr):
    spec = Spec(body=scan(AluOp.ADD, expr), reference=lambda *a: a[0])
    spec_h = _hoist_stream_invariant_ops(spec)
    scans = _collect(spec_h.body, Scan)
    placement = _build_placement(spec_h, scans, N_STAGES["v3"], N_LANES["v3"])
    st = placement.node_stage[scans[0]]
    reset_ov = {st: _Stage(AluOp.ADD, scans[0].expr, Zero)}
    trig_a = (Trigger.SRC_TENSOR_DONE, Trigger.SUB_DIM_DONE, Trigger.COUNT)
    trig_b = (Trigger.SRC_TENSOR_DONE, Trigger.SUB_DIM_DONE, Trigger.NONE)

    def mk(ov, trig, nxt, rep=0):
        return _State(
            placement=placement, consume=(True, False), overrides=ov,
            trigger=trig, next=nxt, repeat=rep,
        )

    states = [
        mk(reset_ov, trig_a, (0, 2, 1), 1),  # entry: reset acc on elem 0
        mk({}, trig_b, (0, 2, 0)),           # steady
        mk(reset_ov, trig_a, (0, 2, 1), 1),  # per-page reset
    ]
    uops = [_assemble(s) for s in states]
    for u in uops:
        u.out_last_subdim_enable = ENABLE
        u.validate("v3")
    return uops, spec


def register_reduce_op():
    for existing in dve_ops.OPS:
        if existing.name == REDUCE_OP_NAME:
            return
    u_reg, spec = _build_variant(Src0)
    u_2x, _ = _build_variant(Src0 + _S0H)
    u_2p, _ = _build_variant(Src0 + Src1)
    u_4x, _ = _build_variant((Src0 + _S0H) + (Src1 + _S1H))
    op = DveOp(REDUCE_OP_NAME, spec, subdim=True, uops_sha={})
    dve_ops.OPS.append(op)
    dve_ops.CUSTOM_DVE_SPECS[REDUCE_OP_NAME] = spec
    dve_ops._SUB_OPCODE_FOR_NAME[REDUCE_OP_NAME] = REDUCE_ROW
    dve_ops._COMPILE_CACHE[(REDUCE_OP_NAME, "v3")] = DveOpSpec(
        name=REDUCE_OP_NAME, opcode=REDUCE_ROW,
        uops=u_reg, uops_2x=u_2x, uops_2x_2p=u_2p, uops_4x=u_4x,
        perf_max=3, rd1_en=False,
    )


# --------------------------------- hijacked fused absdiff (row 0x41, TT)
# TENSOR_TENSOR's row runs |in0 - in1| instead. TT is 2-source, so the
# engine only ever selects REGULAR or 2X_1PORT - both variants provided.
# NOTE: stock nc.vector.tensor_tensor must NOT be used in this kernel
# (use nc.gpsimd.tensor_tensor for small elementwise work).
ABSDIFF_OP_NAME = "ABSDIFF_TT_ANT"
ABSDIFF_ROW = 0x41

from concourse.dve_spec import maxx as _maxx
from concourse.dve_uop import OutPath, OutSel, DELAY_OUT, AluInp, DelayInp, DISABLE


def _build_absdiff_regular():
    body = _maxx(Src0 - Src1, Src1 - Src0)
    spec = Spec(body=body, reference=lambda *a: np.abs(a[0] - a[1]))
    spec_h = _hoist_stream_invariant_ops(spec)
    placement = _build_placement(spec_h, [], N_STAGES["v3"], N_LANES["v3"])
    from concourse.dve_spec import _build_state_machine as _bsm

    states = _bsm(spec_h, [], [], placement)
    uops = [_assemble(s) for s in states]
    for u in uops:
        u.validate("v3")
    return uops, spec


def _build_absdiff_2x():
    ab_lo = _maxx(Src0 - Src1, Src1 - Src0)
    ab_hi = _maxx(_S0H - _S1H, _S1H - _S0H)
    body = ab_hi + ab_lo * Zero
    spec = Spec(body=body, reference=lambda *a: np.abs(a[0] - a[1]))
    spec_h = _hoist_stream_invariant_ops(spec)
    placement = _build_placement(spec_h, [], N_STAGES["v3"], N_LANES["v3"])
    from concourse.dve_spec import _build_state_machine as _bsm

    states = _bsm(spec_h, [], [], placement)
    assert len(states) == 1
    uop = _assemble(states[0])
    # Route: WR0_LO <- ab_lo via a spare delay lane; WR0_HI <- final ALU
    # (= ab_hi, since body adds ab_lo*0).
    prod_stage = placement.node_stage[ab_lo]  # ALU block holding ab_lo
    n_lanes = N_LANES["v3"]
    dps = uop.datapath_config
    free = None
    for lane in range(n_lanes):
        if all(dps[blk].delay_enable[lane] == DISABLE for blk in range(prod_stage + 1, 8)):
            free = lane
            break
    assert free is not None, "no spare delay lane for ab_lo"
    dps[prod_stage + 1].enable_delay_from_src(DelayInp.PREV_ALU_OUT, free)
    for blk in range(prod_stage + 2, 8):
        dps[blk].pass_through_delay(free)
    uop.out[OutPath.WR0_LO] = DELAY_OUT[free]
    uop.out_enable[OutPath.WR0_LO] = ENABLE
    uop.out[OutPath.WR0_HI] = OutSel.ALU_OUT
    uop.out_enable[OutPath.WR0_HI] = ENABLE
    uop.validate("v3")
    return [uop], spec


def register_absdiff_op():
    for existing in dve_ops.OPS:
        if existing.name == ABSDIFF_OP_NAME:
            return
    u_reg, spec = _build_absdiff_regular()
    u_2x, _ = _build_absdiff_2x()
    op = DveOp(ABSDIFF_OP_NAME, spec, subdim=False, uops_sha={})
    dve_ops.OPS.append(op)
    dve_ops.CUSTOM_DVE_SPECS[ABSDIFF_OP_NAME] = spec
    dve_ops._SUB_OPCODE_FOR_NAME[ABSDIFF_OP_NAME] = ABSDIFF_ROW
    dve_ops._COMPILE_CACHE[(ABSDIFF_OP_NAME, "v3")] = DveOpSpec(
        name=ABSDIFF_OP_NAME, opcode=ABSDIFF_ROW,
        uops=u_reg, uops_2x=u_2x, rd1_en=True,
    )


def emit_absdiff(nc, engine, *, out, in0, in1):
    """out = |in0 - in1| via the hijacked TENSOR_TENSOR row."""
    inst = mybir.InstTensorTensor(
        name=nc.get_next_instruction_name(),
        op=mybir.AluOpType.subtract,
        ins=[engine.lower_ap(in0, opt=False), engine.lower_ap(in1, opt=False)],
        outs=[engine.lower_ap(out, opt=False)],
    )
    return engine.add_instruction(inst)


def emit_tree_add(nc, engine, *, out, a, b):
    """Elementwise add via DVE scalar_tensor_tensor (row 0x9d, not hijacked):
    out = (a + 0.0) + b."""
    nc.vector.scalar_tensor_tensor(
        out=out, in0=a, scalar=0.0, in1=b,
        op0=mybir.AluOpType.add, op1=mybir.AluOpType.add,
    )


def emit_seg_sum_reduce(nc, engine, *, out, in0):
    """out[p, pages] (fp32) = per-page sums of in0[p, pages, n] (bf16)."""
    inst = mybir.InstTensorReduce(
        name=nc.get_next_instruction_name(),
        op=mybir.AluOpType.add,
        axis=mybir.AxisListType.X,
        apply_absolute_value=False,
        ins=[engine.lower_ap(in0, opt=False)],
        outs=[engine.lower_ap(out, opt=False)],
    )
    return engine.add_instruction(inst)


# ------------------------------------------------------------ kernel build
_BUILT = None

ROUND = 8  # i-blocks per exp round


def build_bass():
    _patch_tile_drain()
    register_reduce_op()
    register_absdiff_op()
    nc = bass.Bass()
    f32, bf16 = mybir.dt.float32, mybir.dt.bfloat16

    # host-prepared: xt = x^T per set (bf16), wt = W^T (bf16)
    xt_in = nc.declare_dram_parameter("xt", [SETS_PER_CORE, M_DIM, S], bf16, isOutput=False)
    wt_in = nc.declare_dram_parameter("wt", [M_DIM, NO], bf16, isOutput=False)
    out_d = nc.declare_dram_parameter("out", [SETS_PER_CORE, S * O], f32, isOutput=True)

    nc.m.ant_custom_dve_ops = sorted(
        set(nc.m.ant_custom_dve_ops or []) | {REDUCE_OP_NAME, ABSDIFF_OP_NAME}
    )

    MC = M_DIM // 128  # 8 m-chunks
    NPB = 8  # n-groups per psum tile (4 banks)

    with tile.TileContext(nc) as tc:
        with (
            tc.tile_pool(name="const", bufs=1) as constp,
            tc.tile_pool(name="xt", bufs=2) as xtp,
            tc.tile_pool(name="trep", bufs=2) as trepp,
            tc.tile_pool(name="uall", bufs=2) as uallp,
            tc.tile_pool(name="work", bufs=2) as workp,
            tc.tile_pool(name="dtile", bufs=2) as dtp,
            tc.tile_pool(name="etile", bufs=2) as etp,
            tc.tile_pool(name="res", bufs=2) as resp,
            tc.tile_pool(name="ppsum", bufs=2, space="PSUM") as ppsum,
        ):
            wt = [
                constp.tile([128, NO], bf16, tag=f"wt{mc}", name=f"wt{mc}")
                for mc in range(MC)
            ]
            for mc in range(MC):
                nc.sync.dma_start(
                    out=wt[mc][:], in_=wt_in[mc * 128 : (mc + 1) * 128, :]
                )

            for b in range(SETS_PER_CORE):
                # ---- load all xT chunks in one DMA: xtile[p, mc, k]
                xtile = xtp.tile([128, MC, S], bf16, tag="xtile")
                xb_ap = xt_in[b]  # [M, S]
                src = bass.AP(
                    tensor=xb_ap.tensor, offset=xb_ap.offset,
                    ap=[[S, 128], [128 * S, MC], [1, S]],
                )
                nc.sync.dma_start(out=xtile[:], in_=src)

                # ---- projection into T_rep [p=(r4,o32), k=S, n=N] bf16
                # partitions [0:32] computed, then replicated via SBUF DMA.
                trep = trepp.tile([128, S, N], bf16)
                for ng in range(N // NPB):  # 4 psum tiles of 8 n each
                    ps = ppsum.tile([32, NPB * S], f32, tag="proj")
                    for j in range(NPB):
                        n = ng * NPB + j
                        for mc in range(MC):
                            nc.tensor.matmul(
                                ps[:, j * S : (j + 1) * S],
                                wt[mc][:, n * O : (n + 1) * O],
                                xtile[:, mc, :],
                                start=(mc == 0), stop=(mc == MC - 1),
                                skip_group_check=True,
                            )
                    # evac 8 n-slices -> T_rep[0:32, :, ng*8 : ng*8+8]
                    tr_ap = trep[0:32, :, :]
                    dst = bass.AP(
                        tensor=tr_ap.tensor,
                        offset=tr_ap.offset + ng * NPB,
                        ap=[list(tr_ap.ap[0]), [1, NPB], [N, S]],
                    )
                    nc.scalar.copy(out=dst, in_=ps[:])
                for r in range(1, IB):
                    nc.sync.dma_start(
                        out=trep[32 * r : 32 * (r + 1), :, :],
                        in_=trep[0:32, :, :],
                    )

                # ---- U_all [p=(r4,o32), blk=64, n] bf16
                uall = uallp.tile([128, NBLK, N], bf16)
                for r in range(IB):
                    src3 = trep[32 * r : 32 * (r + 1), :, :]
                    usrc = bass.AP(
                        tensor=src3.tensor,
                        offset=src3.offset + r * N,
                        ap=[list(src3.ap[0]), [IB * N, NBLK], [1, N]],
                    )
                    nc.vector.tensor_copy(uall[32 * r : 32 * (r + 1), :, :], usrc)

                outt = resp.tile([128, NBLK], f32, tag="outt")

                # ---- main loop: rounds of ROUND i-blocks, full k (no triangle)
                for rnd in range(NBLK // ROUND):
                    dvals = dtp.tile([128, ROUND, S], f32, tag="dvals")
                    for j in range(ROUND):
                        blk = rnd * ROUND + j
                        diff = workp.tile([128, S, N], bf16, tag="diff")
                        u_ap = uall[:]
                        in1 = bass.AP(
                            tensor=u_ap.tensor,
                            offset=u_ap.offset + blk * N,
                            ap=[list(u_ap.ap[0]), [0, S], [1, N]],
                        )
                        emit_absdiff(
                            nc, nc.vector, out=diff[:], in0=trep[:], in1=in1
                        )
                        # n-reduction: tree of adds over the inner dim 32
                        w = N // 2
                        while w >= 1:
                            dst = (
                                diff[:, :, 0:w]
                                if w > 1
                                else dvals[:, j, :].unsqueeze(2)
                            )
                            nc.gpsimd  # noqa - placeholder no-op attr access
                            emit_tree_add(
                                nc, nc.vector,
                                out=dst,
                                a=diff[:, :, 0:w],
                                b=diff[:, :, w : 2 * w],
                            )
                            w //= 2
                    # exp(-d) for the whole round on ScalarE (fp32 out)
                    etile_t = etp.tile([128, ROUND * S], f32, tag="etile")
                    dflat = bass.AP(
                        tensor=dvals[:].tensor, offset=dvals[:].offset,
                        ap=[list(dvals[:].ap[0]), [1, ROUND * S]],
                    )
                    nc.scalar.activation(
                        out=etile_t[:], in_=dflat,
                        func=mybir.ActivationFunctionType.Exp, scale=-1.0,
                    )
                    # row sums per block: tree over k (fp32)
                    ev = etile_t[:].rearrange("p (r s) -> p r s", r=ROUND)
                    w = S // 2
                    while w >= 1:
                        dst = (
                            ev[:, :, 0:w]
                            if w > 1
                            else outt[:, rnd * ROUND : (rnd + 1) * ROUND].unsqueeze(2)
                        )
                        emit_tree_add(
                            nc, nc.vector,
                            out=dst, a=ev[:, :, 0:w], b=ev[:, :, w : 2 * w],
                        )
                        w //= 2

                # ---- DMA out: element (p, blk) -> out[b, 128*blk + p]
                od = out_d[b, :]
                dst = bass.AP(
                    tensor=od.tensor, offset=od.offset,
                    ap=[[1, 128], [128, NBLK]],
                )
                nc.sync.dma_start(out=dst, in_=outt[:])

    return nc


def _get_built():
    global _BUILT
    if _BUILT is None:
        _BUILT = build_bass()
    return _BUILT


# ------------------------------------------------------------- entry point
TRACE = False
LAST_RESULTS = None


def kernel(x: np.ndarray, W: np.ndarray) -> np.ndarray:
    global LAST_RESULTS
    import ml_dtypes
    from concourse.bass_utils import run_bass_kernel_spmd

    nc = _get_built()
    bf = ml_dtypes.bfloat16

    Wb = np.asarray(W, np.float32).astype(bf)
    wt_host = np.ascontiguousarray(Wb.T)  # [M, NO]

    xb = np.asarray(x, np.float32).astype(bf)  # [B, S, M]
    in_maps = []
    for c in range(NCORES):
        xs = xb[c * SETS_PER_CORE : (c + 1) * SETS_PER_CORE]  # [4, S, M]
        xt = np.ascontiguousarray(np.swapaxes(xs, 1, 2))  # [4, M, S]
        in_maps.append({"xt": xt, "wt": wt_host})

    res = run_bass_kernel_spmd(nc, in_maps, list(range(NCORES)), trace=TRACE)
    LAST_RESULTS = res
    outs = [res.results[c]["out"] for c in range(NCORES)]
    return np.concatenate(outs, axis=0).reshape(B, S * O)

